# revision 24
# baseline (speedup 1.0000x reference)
"""Trainium2 Bass kernel for nn_MultiHeadAttention_90924457656943.

Strategy (8 NeuronCores, SPMD):
  - Row-shard the 2048 (b,s) token rows: core j owns rows [256j, 256j+256).
    Each core computes q/k/v for its rows with FULL weights (double
    projections done in transposed space: y^T = W^T @ x^T so weight chunks
    are the stationary matmul operand in natural layout).
  - AllToAll redistributes q/k/v so core j owns feature columns
    [128j, 128j+128) = effective heads [8j, 8j+8) for the full 2048 rows.
    q/k are shipped transposed (head-dim on partitions) which is exactly
    the layout attention needs; v ships natural.
  - Attention per (head, batch) with a flash-style loop: scores^T matmul
    (contraction over head_dim=16, packed 2 heads per pass into 32-row
    strips of the PE array), exp on ScalarE directly out of PSUM (the
    1/sqrt(16) scale folded into the activation's free affine), then
    attn@v with a ones-column appended to v so the softmax denominators
    fall out of the same matmul.
  - The module's quirky head-merge (torch .view after concat) is a fixed
    permutation; it maps each core's heads exactly onto o_perm columns
    [128j, 128j+128).  Constant 0/1 permutation matrices on the PE build
    the transposed o_perm^T slab directly.
  - AllToAll redistributes o_perm^T back to row shards; final projection
    runs transposed (out^T = wo^T @ o_perm^T) so the wo bias is a
    per-partition add.  Host transposes each core's (1024, 256) output
    shard back into rows.
  - All matmuls use float32r (TF32-like, full PE rate at N>=256); inputs
    are pre-rounded on the host.
"""

import os
import numpy as np

import concourse.bass as bass
import concourse.tile as tile
from concourse import bacc, mybir
from concourse.bass_utils import run_bass_kernel_spmd

F32 = mybir.dt.float32
F32R = mybir.dt.float32r
AF = mybir.ActivationFunctionType

B, S, F = 2, 1024, 1024
H = 16          # head dim
C = 64          # effective heads
NCORES = 8
ROWS = (B * S) // NCORES          # 256 token rows per core
KC = F // 128                     # 8 contraction chunks
HEADS_PER_CORE = C // NCORES      # 8


def _round_tf32(x: np.ndarray) -> np.ndarray:
    """Round fp32 to the PE's fp32r (tf32-like) format: RNE to 12 dropped bits."""
    u = np.ascontiguousarray(x, dtype=np.float32).view(np.uint32).copy()
    lsb = (u >> 12) & 1
    u += 0x7FF + lsb
    u &= np.uint32(0xFFFFF000)
    return u.view(np.float32)


def _perm_mats() -> np.ndarray:
    """32 constant matrices P[v,r,u]: rows 32v+h -> cols 64u+16r+h."""
    P = np.zeros((4, 4, 2, 128, 128), dtype=np.float32)
    for v in range(4):
        for r in range(4):
            for u in range(2):
                for h in range(H):
                    P[v, r, u, 32 * v + h, 64 * u + 16 * r + h] = 1.0
    return P.reshape(32, 128, 128)


WNAMES = ("wq_w", "vq_w", "wk_w", "vk_w", "wv_w", "vv_w", "wo_w")
BNAMES = ("wq_b", "vq_b", "wk_b", "vk_b", "wv_b", "vv_b", "wo_b")


def _build():
    nc = bacc.Bacc("TRN2", target_bir_lowering=False, debug=False,
                   num_devices=NCORES)

    xT = nc.dram_tensor("xT", [F, ROWS], F32R, kind="ExternalInput")
    W = {n: nc.dram_tensor(n, [KC, KC, 128, 128], F32R, kind="ExternalInput")
         for n in WNAMES}
    Bv = {n: nc.dram_tensor(n, [F], F32, kind="ExternalInput") for n in BNAMES}
    perm = nc.dram_tensor("perm", [32, 128, 128], F32R, kind="ExternalInput")
    outT = nc.dram_tensor("outT", [F, ROWS], F32, kind="ExternalOutput")

    # internal DRAM: A2A bounce buffers + reciprocal broadcast bounce
    a2aq_in = nc.dram_tensor("a2aq_in", [NCORES, 128, ROWS], F32R)
    a2aq_out = nc.dram_tensor("a2aq_out", [NCORES, 128, ROWS], F32R)
    a2ak_in = nc.dram_tensor("a2ak_in", [NCORES, 128, ROWS], F32R)
    a2ak_out = nc.dram_tensor("a2ak_out", [NCORES, 128, ROWS], F32R)
    a2av_in = nc.dram_tensor("a2av_in", [NCORES, ROWS, 128], F32R)
    a2av_out = nc.dram_tensor("a2av_out", [NCORES, ROWS, 128], F32R)
    a2ao_in = nc.dram_tensor("a2ao_in", [NCORES, 128, ROWS], F32R)
    a2ao_out = nc.dram_tensor("a2ao_out", [NCORES, 128, ROWS], F32R)
    rec_dram = nc.dram_tensor("rec_dram", [2, 8, S], F32)   # (b2, 4u+v... flat row, s)
    rec2_dram = nc.dram_tensor("rec2_dram", [2, 8, S], F32)  # reciprocals

    RG = [list(range(NCORES))]

    def a2a(dst, src):
        nc.gpsimd.collective_compute(
            "AllToAll", mybir.AluOpType.bypass,
            ins=[src[:]], outs=[dst[:]], replica_groups=RG)

    from contextlib import ExitStack
    with tile.TileContext(nc) as tc, ExitStack() as _stk:
        # ---------- persistent pools ----------
        const_pool = _stk.enter_context(tc.tile_pool(name="const", bufs=1))
        # biases as (128, 8) column tiles
        bcol = {}
        for n in BNAMES:
            t = const_pool.tile([128, KC], F32, tag=f"b_{n}")
            nc.gpsimd.dma_start(out=t[:], in_=Bv[n].ap().rearrange(
                "(m p) -> p m", p=128))
            bcol[n] = t
        # b2v broadcast across partitions (for the natural-layout v bias add)
        b2v_bc = const_pool.tile([128, F], F32, tag="b2v_bc")
        nc.gpsimd.dma_start(out=b2v_bc[:], in_=Bv["vv_b"].ap().partition_broadcast(128))

        # ---------- phase 1: projections ----------
        ppsum = _stk.enter_context(tc.tile_pool(name="ppsum", bufs=2, space="PSUM"))
        with tc.tile_pool(name="wpool", bufs=8) as wpool, \
             tc.tile_pool(name="wvpool", bufs=2) as wvpool, \
             tc.tile_pool(name="ypool", bufs=1) as ypool, \
             tc.tile_pool(name="stage", bufs=2) as stage:

            # x^T resident tiles (released with this pool after v's y1)
            xt = []
            for k in range(KC):
                t = ypool.tile([128, ROWS], F32R, tag=f"xt{k}")
                nc.gpsimd.dma_start(out=t[:], in_=xT[128 * k:128 * (k + 1), :])
                xt.append(t)

            def projT(wname, bname, rhs_tiles, ytag):
                """y^T[mchunk] = sum_k W[k,m]^T-style matmul + bias; returns tiles."""
                out_tiles = []
                for m in range(KC):
                    ps = ppsum.tile([128, ROWS], F32, tag="pp")
                    for k in range(KC):
                        wt = wpool.tile([128, 128], F32R, tag="w")
                        nc.gpsimd.dma_start(out=wt[:], in_=W[wname][k, m])
                        nc.tensor.matmul(ps[:], wt[:], rhs_tiles[k][:],
                                         start=(k == 0), stop=(k == KC - 1))
                    ot = ypool.tile([128, ROWS], F32R, tag=f"{ytag}{m}")
                    nc.vector.tensor_scalar_add(ot[:], ps[:], bcol[bname][:, m:m + 1])
                    out_tiles.append(ot)
                return out_tiles

            # q and k: two transposed projections, ship transposed
            for wn1, bn1, wn2, bn2, dst in (
                    ("wq_w", "wq_b", "vq_w", "vq_b", a2aq_in),
                    ("wk_w", "wk_b", "vk_w", "vk_b", a2ak_in)):
                y1 = projT(wn1, bn1, xt, "y1")
                y2 = projT(wn2, bn2, y1, "y2")
                for m in range(KC):
                    nc.gpsimd.dma_start(out=dst[m], in_=y2[m][:])
                if dst is a2aq_in:
                    a2a(a2aq_out, a2aq_in)
                else:
                    a2a(a2ak_out, a2ak_in)

            # v: first projection transposed, second natural
            y1v = projT("wv_w", "wv_b", xt, "y1")
            for mb in range(ROWS // 128):          # bs chunk
                for n2 in range(F // 512):          # f_out 512-chunk
                    ps = ppsum.tile([128, 512], F32, tag="pp")
                    for k in range(KC):
                        wt = wvpool.tile([128, 512], F32R, tag="wv")
                        nc.gpsimd.dma_start(
                            out=wt[:].rearrange("p (m f) -> p m f", m=4),
                            in_=W["vv_w"][k, 4 * n2:4 * (n2 + 1)].transpose(
                                [1, 0, 2]))
                        nc.tensor.matmul(
                            ps[:], y1v[k][:, 128 * mb:128 * (mb + 1)], wt[:],
                            start=(k == 0), stop=(k == KC - 1))
                    ot = stage.tile([128, 512], F32R, tag="vout")
                    nc.vector.tensor_add(ot[:], ps[:],
                                         b2v_bc[:, 512 * n2:512 * (n2 + 1)])
                    # scatter the 4 128-col chunks to their a2a slots
                    for mm in range(4):
                        nc.gpsimd.dma_start(
                            out=a2av_in[4 * n2 + mm,
                                        128 * mb:128 * (mb + 1), :],
                            in_=ot[:, 128 * mm:128 * (mm + 1)])
            a2a(a2av_out, a2av_in)

        # ---------- phase 2: attention ----------
        # (b2, half) -> packed unnormalized o^T tile: head cl at rows 32*(cl%4)
        onpool = _stk.enter_context(tc.tile_pool(name="on", bufs=1))
        on_tiles = {}
        for _b2 in range(2):
            for _hf in range(2):
                on_t = onpool.tile([128, S], F32R, tag=f"on{2 * _b2 + _hf}")
                on_tiles[(_b2, _hf)] = on_t
        with tc.tile_pool(name="qk", bufs=2) as qkpool, \
             tc.tile_pool(name="vt", bufs=10) as vtpool, \
             tc.tile_pool(name="ex", bufs=2) as expool, \
             tc.tile_pool(name="dn", bufs=2) as dnpool, \
             tc.tile_pool(name="scp", bufs=2, space="PSUM") as scpsum, \
             tc.tile_pool(name="avp", bufs=1, space="PSUM") as avpsum:

            for b2 in range(2):
                for g in range(4):          # 2-head groups: heads 2g, 2g+1
                    qs = qkpool.tile([128, S], F32R, tag="qs")
                    ks = qkpool.tile([128, S], F32R, tag="ks")
                    for i in range(4):      # bs chunk within batch b2
                        ci = 4 * b2 + i
                        for m in range(2):
                            cl = 2 * g + m
                            nc.gpsimd.dma_start(
                                out=qs[32 * m:32 * m + 16, 256 * i:256 * (i + 1)],
                                in_=a2aq_out[ci, 16 * cl:16 * cl + 16, :])
                            nc.gpsimd.dma_start(
                                out=ks[32 * m:32 * m + 16, 256 * i:256 * (i + 1)],
                                in_=a2ak_out[ci, 16 * cl:16 * cl + 16, :])
                    # v tiles with ones column: (128, 34) per s_k chunk
                    vts = []
                    for kc in range(8):
                        vt = vtpool.tile([128, 34], F32R, tag="vones")
                        ci = 4 * b2 + kc // 2
                        half = kc % 2
                        nc.gpsimd.dma_start(
                            out=vt[:].rearrange("p (m f) -> p m f",
                                                f=17)[:, :, 0:16],
                            in_=a2av_out[ci, 128 * half:128 * (half + 1),
                                         32 * g:32 * (g + 1)].rearrange(
                                             "p (m f) -> p m f", m=2))
                        nc.gpsimd.memset(vt[:, 16::17].bitcast(F32), 1.0)
                        vts.append(vt)

                    # stage: (17, [head m][q2][512]) unnormalized o^T + denoms
                    stg = dnpool.tile([17, 2 * S], F32R, tag="stg")
                    stg4 = stg[:].rearrange("p (m q f) -> p m q f", m=2, q=2)
                    for q2 in range(2):
                        av = avpsum.tile([17, 1024], F32, tag="av")
                        for kc in range(8):
                            sc = scpsum.tile([128, 1024], F32, tag="sc")
                            for m in range(2):
                                nc.tensor.matmul(
                                    sc[:, 512 * m:512 * (m + 1)],
                                    ks[32 * m:32 * m + 16,
                                       128 * kc:128 * (kc + 1)],
                                    qs[32 * m:32 * m + 16,
                                       512 * q2:512 * (q2 + 1)],
                                    start=True, stop=True,
                                    tile_position=(32 * m, 0))
                            ex = expool.tile([128, 1024], F32R, tag="ex")
                            nc.scalar.activation(ex[:], sc[:], AF.Exp, scale=0.25)
                            for m in range(2):
                                nc.tensor.matmul(
                                    av[:, 512 * m:512 * (m + 1)],
                                    vts[kc][:, 17 * m:17 * (m + 1)],
                                    ex[:, 512 * m:512 * (m + 1)],
                                    start=(kc == 0), stop=(kc == 7),
                                    skip_group_check=True)
                        nc.vector.tensor_copy(
                            stg4[:, :, q2, :],
                            av[:].rearrange("p (m f) -> p m f", m=2))
                    # export denominator rows; scatter o^T into packed tiles
                    for m in range(2):
                        cl = 2 * g + m
                        nc.gpsimd.dma_start(out=rec_dram[b2, cl],
                                            in_=stg4[16:17, m])
                        on = on_tiles[(b2, cl // 4)]
                        nc.gpsimd.dma_start(
                            out=on[32 * (cl % 4):32 * (cl % 4) + 16, :],
                            in_=stg4[0:16, m])

                # batched reciprocal of this batch-half's 8 denominator rows
                rt = dnpool.tile([8, S], F32, tag="rt")
                nc.gpsimd.dma_start(out=rt[:], in_=rec_dram[b2])
                rt2 = dnpool.tile([8, S], F32, tag="rt2")
                nc.vector.reciprocal(rt2[:], rt[:])
                nc.gpsimd.dma_start(out=rec2_dram[b2], in_=rt2[:])

        # ---------- phase 3: permutation + A2A + output projection ----------
        with tc.tile_pool(name="po", bufs=4) as popool, \
             tc.tile_pool(name="wo", bufs=16) as wopool, \
             tc.tile_pool(name="oo", bufs=1) as oopool:

            perm_sb = popool.tile([128, 32 * 128], F32R, tag="perm")
            nc.gpsimd.dma_start(
                out=perm_sb[:].rearrange("p (n f) -> p n f", n=32),
                in_=perm.ap().transpose([1, 0, 2]))

            def psl(i):  # perm matrix slice index -> lhsT AP
                return perm_sb[:, 128 * i:128 * (i + 1)]

            for b2 in range(2):
                for v_ in range(4):
                    jp = ((v_ >> 1) & 1) * 4 + (v_ & 1) * 2 + b2
                    ps = ppsum.tile([128, 256], F32, tag="pp")
                    nmm = 0
                    for u in range(2):
                        src = on_tiles[(b2, u)]      # head cl=4u+v_ at rows 32v_
                        for r in range(4):
                            pi = (v_ * 4 + r) * 2 + u
                            nc.tensor.matmul(
                                ps[:],
                                psl(pi)[32 * v_:32 * v_ + 16, :],
                                src[32 * v_:32 * v_ + 16, r::4],
                                start=(nmm == 0), stop=(nmm == 7),
                                tile_position=(32 * v_, 0),
                                skip_group_check=True)
                            nmm += 1
                    # gathered reciprocal: M[64u+16r+h, a] = 1/denom[4u+v_, 4a+r]
                    mt = popool.tile([128, 256], F32, tag="mt")
                    for u in range(2):
                        for r in range(4):
                            src_ap = bass.AP(
                                tensor=rec2_dram,
                                offset=(8 * b2 + 4 * u + v_) * S + r,
                                ap=[[0, 16], [4, 256]])
                            nc.gpsimd.dma_start(
                                out=mt[64 * u + 16 * r:64 * u + 16 * (r + 1), :],
                                in_=src_ap)
                    ot = popool.tile([128, 256], F32R, tag="pout")
                    nc.vector.tensor_mul(ot[:], ps[:], mt[:])
                    nc.gpsimd.dma_start(out=a2ao_in[jp], in_=ot[:])
            a2a(a2ao_out, a2ao_in)

            rhs_o = []
            for k in range(KC):
                t = oopool.tile([128, ROWS], F32R, tag=f"ro{k}")
                nc.gpsimd.dma_start(out=t[:], in_=a2ao_out[k])
                rhs_o.append(t)
            for n in range(KC):
                ps = ppsum.tile([128, ROWS], F32, tag="pp")
                for k in range(KC):
                    wt = wopool.tile([128, 128], F32R, tag="wo")
                    nc.gpsimd.dma_start(out=wt[:], in_=W["wo_w"][k, n])
                    nc.tensor.matmul(ps[:], wt[:], rhs_o[k][:],
                                     start=(k == 0), stop=(k == KC - 1))
                ot = oopool.tile([128, ROWS], F32, tag="fout")
                nc.vector.tensor_scalar_add(ot[:], ps[:], bcol["wo_b"][:, n:n + 1])
                nc.gpsimd.dma_start(out=outT[128 * n:128 * (n + 1), :], in_=ot[:])

    nc.finalize()
    return nc


_NC_CACHE = None


def _get_nc():
    global _NC_CACHE
    if _NC_CACHE is None:
        _NC_CACHE = _build()
    return _NC_CACHE


def kernel(x, wq_w, wq_b, wk_w, wk_b, wv_w, wv_b,
           vq_w, vq_b, vk_w, vk_b, vv_w, vv_b, wo_w, wo_b,
           _trace=False):
    nc = _get_nc()

    ws = {"wq_w": wq_w, "vq_w": vq_w, "wk_w": wk_w, "vk_w": vk_w,
          "wv_w": wv_w, "vv_w": vv_w, "wo_w": wo_w}
    bs = {"wq_b": wq_b, "vq_b": vq_b, "wk_b": wk_b, "vk_b": vk_b,
          "wv_b": wv_b, "vv_b": vv_b, "wo_b": wo_b}

    wchunks = {n: np.ascontiguousarray(
        _round_tf32(np.asarray(w)).reshape(KC, 128, KC, 128).transpose(0, 2, 1, 3))
        for n, w in ws.items()}
    bmap = {n: np.ascontiguousarray(np.asarray(b, dtype=np.float32))
            for n, b in bs.items()}
    P = _round_tf32(_perm_mats())

    xf = np.asarray(x, dtype=np.float32).reshape(B * S, F)
    in_maps = []
    for j in range(NCORES):
        xTj = np.ascontiguousarray(
            _round_tf32(xf[ROWS * j:ROWS * (j + 1)]).T)
        m = {"xT": xTj, "perm": P}
        m.update(wchunks)
        m.update(bmap)
        in_maps.append(m)

    kw = {}
    if _trace:
        import sys
        import types
        if "antenv.axon_hooks" not in sys.modules:
            import antenv
            mod = types.ModuleType("antenv.axon_hooks")
            mod._hook = None
            def _set(h):
                mod._hook = h
            def _get():
                return mod._hook
            mod.set_axon_ntff_profile_hook = _set
            mod.get_axon_ntff_profile_hook = _get
            sys.modules["antenv.axon_hooks"] = mod
            antenv.axon_hooks = mod
            from trn_agent_boot.trn_boot import _ntff_profile_via_ctypes
            _set(_ntff_profile_via_ctypes("/opt/axon/libaxon_pjrt.so"))
        kw = dict(trace=True, trace_cores=list(range(NCORES)))
    res = run_bass_kernel_spmd(nc, in_maps, core_ids=list(range(NCORES)), **kw)

    out = np.empty((B * S, F), dtype=np.float32)
    for j in range(NCORES):
        out[ROWS * j:ROWS * (j + 1)] = res.results[j]["outT"].T
    if _trace:
        return out.reshape(B, S, F), res
    return out.reshape(B, S, F)


# revision 26
# speedup vs baseline: 1.1321x; 1.1321x over previous
"""Trainium2 Bass kernel for nn_MultiHeadAttention_90924457656943.

Strategy (8 NeuronCores, SPMD):
  - Row-shard the 2048 (b,s) token rows: core j owns rows [256j, 256j+256).
    Each core computes q/k/v for its rows with FULL weights (double
    projections done in transposed space: y^T = W^T @ x^T so weight chunks
    are the stationary matmul operand in natural layout).
  - AllToAll redistributes q/k/v so core j owns feature columns
    [128j, 128j+128) = effective heads [8j, 8j+8) for the full 2048 rows.
    q/k are shipped transposed (head-dim on partitions) which is exactly
    the layout attention needs; v ships natural.
  - Attention per (head, batch) with a flash-style loop: scores^T matmul
    (contraction over head_dim=16, packed 2 heads per pass into 32-row
    strips of the PE array), exp on ScalarE directly out of PSUM (the
    1/sqrt(16) scale folded into the activation's free affine), then
    attn@v with a ones-column appended to v so the softmax denominators
    fall out of the same matmul.
  - The module's quirky head-merge (torch .view after concat) is a fixed
    permutation; it maps each core's heads exactly onto o_perm columns
    [128j, 128j+128).  Constant 0/1 permutation matrices on the PE build
    the transposed o_perm^T slab directly.
  - AllToAll redistributes o_perm^T back to row shards; final projection
    runs transposed (out^T = wo^T @ o_perm^T) so the wo bias is a
    per-partition add.  Host transposes each core's (1024, 256) output
    shard back into rows.
  - All matmuls use float32r (TF32-like, full PE rate at N>=256); inputs
    are pre-rounded on the host.
"""

import os
import numpy as np

import concourse.bass as bass
import concourse.tile as tile
from concourse import bacc, mybir
from concourse.bass_utils import run_bass_kernel_spmd

F32 = mybir.dt.float32
F32R = mybir.dt.float32r
AF = mybir.ActivationFunctionType

B, S, F = 2, 1024, 1024
H = 16          # head dim
C = 64          # effective heads
NCORES = 8
ROWS = (B * S) // NCORES          # 256 token rows per core
KC = F // 128                     # 8 contraction chunks
HEADS_PER_CORE = C // NCORES      # 8


def _round_tf32(x: np.ndarray) -> np.ndarray:
    """Round fp32 to the PE's fp32r (tf32-like) format: RNE to 12 dropped bits."""
    u = np.ascontiguousarray(x, dtype=np.float32).view(np.uint32).copy()
    lsb = (u >> 12) & 1
    u += 0x7FF + lsb
    u &= np.uint32(0xFFFFF000)
    return u.view(np.float32)


def _perm_mats() -> np.ndarray:
    """32 constant matrices P[v,r,u]: rows 32v+h -> cols 64u+16r+h."""
    P = np.zeros((4, 4, 2, 128, 128), dtype=np.float32)
    for v in range(4):
        for r in range(4):
            for u in range(2):
                for h in range(H):
                    P[v, r, u, 32 * v + h, 64 * u + 16 * r + h] = 1.0
    return P.reshape(32, 128, 128)


WNAMES = ("wq_w", "vq_w", "wk_w", "vk_w", "wv_w", "vv_w", "wo_w")
BNAMES = ("wq_b", "vq_b", "wk_b", "vk_b", "wv_b", "vv_b", "wo_b")


def _build():
    nc = bacc.Bacc("TRN2", target_bir_lowering=False, debug=False,
                   num_devices=NCORES)

    xT = nc.dram_tensor("xT", [F, ROWS], F32R, kind="ExternalInput")
    W = {n: nc.dram_tensor(n, [KC, KC, 128, 128], F32R, kind="ExternalInput")
         for n in WNAMES}
    Bv = {n: nc.dram_tensor(n, [F], F32, kind="ExternalInput") for n in BNAMES}
    perm = nc.dram_tensor("perm", [32, 128, 128], F32R, kind="ExternalInput")
    outT = nc.dram_tensor("outT", [F, ROWS], F32, kind="ExternalOutput")

    # internal DRAM: A2A bounce buffers + reciprocal broadcast bounce
    a2aq_in = nc.dram_tensor("a2aq_in", [NCORES, 128, ROWS], F32R)
    a2aq_out = nc.dram_tensor("a2aq_out", [NCORES, 128, ROWS], F32R)
    a2ak_in = nc.dram_tensor("a2ak_in", [NCORES, 128, ROWS], F32R)
    a2ak_out = nc.dram_tensor("a2ak_out", [NCORES, 128, ROWS], F32R)
    a2av_in = nc.dram_tensor("a2av_in", [NCORES, ROWS, 128], F32R)
    a2av_out = nc.dram_tensor("a2av_out", [NCORES, ROWS, 128], F32R)
    a2ao_in = nc.dram_tensor("a2ao_in", [NCORES, 128, ROWS], F32R)
    a2ao_out = nc.dram_tensor("a2ao_out", [NCORES, 128, ROWS], F32R)
    rec_dram = nc.dram_tensor("rec_dram", [2, 8, S], F32)   # (b2, 4u+v... flat row, s)
    rec2_dram = nc.dram_tensor("rec2_dram", [2, 8, S], F32)  # reciprocals

    RG = [list(range(NCORES))]

    def a2a(dst, src):
        nc.gpsimd.collective_compute(
            "AllToAll", mybir.AluOpType.bypass,
            ins=[src[:]], outs=[dst[:]], replica_groups=RG)

    from contextlib import ExitStack
    with tile.TileContext(nc) as tc, ExitStack() as _stk:
        # ---------- persistent pools ----------
        const_pool = _stk.enter_context(tc.tile_pool(name="const", bufs=1))
        # biases as (128, 8) column tiles
        bcol = {}
        for n in BNAMES:
            t = const_pool.tile([128, KC], F32, tag=f"b_{n}")
            nc.sync.dma_start(out=t[:], in_=Bv[n].ap().rearrange(
                "(m p) -> p m", p=128))
            bcol[n] = t
        # b2v broadcast across partitions (for the natural-layout v bias add)
        b2v_bc = const_pool.tile([128, F], F32, tag="b2v_bc")
        nc.sync.dma_start(out=b2v_bc[:], in_=Bv["vv_b"].ap().partition_broadcast(128))

        # ---------- phase 1: projections ----------
        ppsum = _stk.enter_context(tc.tile_pool(name="ppsum", bufs=2, space="PSUM"))
        with tc.tile_pool(name="wpool", bufs=3) as wpool, \
             tc.tile_pool(name="wvpool", bufs=2) as wvpool, \
             tc.tile_pool(name="ypool", bufs=1) as ypool, \
             tc.tile_pool(name="stage", bufs=2) as stage:

            # x^T resident tiles (released with this pool after v's y1)
            xt = []
            for k in range(KC):
                t = ypool.tile([128, ROWS], F32R, tag=f"xt{k}")
                nc.sync.dma_start(out=t[:], in_=xT[128 * k:128 * (k + 1), :])
                xt.append(t)

            def projT(wname, bname, rhs_tiles, ytag):
                """y^T[mchunk] = sum_k W[k,m]^T-style matmul + bias; returns tiles."""
                out_tiles = []
                for m in range(KC):
                    ps = ppsum.tile([128, ROWS], F32, tag="pp")
                    wt = wpool.tile([128, KC * 128], F32R, tag="w")
                    nc.sync.dma_start(
                        out=wt[:].rearrange("p (k f) -> p k f", k=KC),
                        in_=W[wname][:, m].transpose([1, 0, 2]))
                    for k in range(KC):
                        nc.tensor.matmul(ps[:], wt[:, 128 * k:128 * (k + 1)],
                                         rhs_tiles[k][:],
                                         start=(k == 0), stop=(k == KC - 1))
                    ot = ypool.tile([128, ROWS], F32R, tag=f"{ytag}{m}")
                    nc.vector.tensor_scalar_add(ot[:], ps[:], bcol[bname][:, m:m + 1])
                    out_tiles.append(ot)
                return out_tiles

            # q and k: two transposed projections, ship transposed
            for wn1, bn1, wn2, bn2, dst in (
                    ("wq_w", "wq_b", "vq_w", "vq_b", a2aq_in),
                    ("wk_w", "wk_b", "vk_w", "vk_b", a2ak_in)):
                y1 = projT(wn1, bn1, xt, "y1")
                y2 = projT(wn2, bn2, y1, "y2")
                for m in range(KC):
                    nc.sync.dma_start(out=dst[m], in_=y2[m][:])
                if dst is a2aq_in:
                    a2a(a2aq_out, a2aq_in)
                else:
                    a2a(a2ak_out, a2ak_in)

            # v: first projection transposed, second natural
            y1v = projT("wv_w", "wv_b", xt, "y1")
            for mb in range(ROWS // 128):          # bs chunk
                for n2 in range(F // 512):          # f_out 512-chunk
                    ps = ppsum.tile([128, 512], F32, tag="pp")
                    for k in range(KC):
                        wt = wvpool.tile([128, 512], F32R, tag="wv")
                        nc.sync.dma_start(
                            out=wt[:].rearrange("p (m f) -> p m f", m=4),
                            in_=W["vv_w"][k, 4 * n2:4 * (n2 + 1)].transpose(
                                [1, 0, 2]))
                        nc.tensor.matmul(
                            ps[:], y1v[k][:, 128 * mb:128 * (mb + 1)], wt[:],
                            start=(k == 0), stop=(k == KC - 1))
                    ot = stage.tile([128, 512], F32R, tag="vout")
                    nc.vector.tensor_add(ot[:], ps[:],
                                         b2v_bc[:, 512 * n2:512 * (n2 + 1)])
                    # scatter the 4 128-col chunks to their a2a slots
                    for mm in range(4):
                        nc.sync.dma_start(
                            out=a2av_in[4 * n2 + mm,
                                        128 * mb:128 * (mb + 1), :],
                            in_=ot[:, 128 * mm:128 * (mm + 1)])
            a2a(a2av_out, a2av_in)

        # ---------- phase 2: attention ----------
        # (b2, half) -> packed unnormalized o^T tile: head cl at rows 32*(cl%4)
        onpool = _stk.enter_context(tc.tile_pool(name="on", bufs=1))
        on_tiles = {}
        for _b2 in range(2):
            for _hf in range(2):
                on_t = onpool.tile([128, S], F32R, tag=f"on{2 * _b2 + _hf}")
                on_tiles[(_b2, _hf)] = on_t
        with tc.tile_pool(name="qk", bufs=2) as qkpool, \
             tc.tile_pool(name="vt", bufs=10) as vtpool, \
             tc.tile_pool(name="ex", bufs=2) as expool, \
             tc.tile_pool(name="dn", bufs=2) as dnpool, \
             tc.tile_pool(name="scp", bufs=2, space="PSUM") as scpsum, \
             tc.tile_pool(name="avp", bufs=1, space="PSUM") as avpsum:

            for b2 in range(2):
                for g in range(4):          # 2-head groups: heads 2g, 2g+1
                    qs = qkpool.tile([128, S], F32R, tag="qs")
                    ks = qkpool.tile([128, S], F32R, tag="ks")
                    for m in range(2):
                        cl = 2 * g + m
                        nc.sync.dma_start(
                            out=qs[32 * m:32 * m + 16, :].rearrange(
                                "p (i f) -> p i f", i=4),
                            in_=a2aq_out[4 * b2:4 * (b2 + 1),
                                         16 * cl:16 * cl + 16, :].transpose(
                                             [1, 0, 2]))
                        nc.sync.dma_start(
                            out=ks[32 * m:32 * m + 16, :].rearrange(
                                "p (i f) -> p i f", i=4),
                            in_=a2ak_out[4 * b2:4 * (b2 + 1),
                                         16 * cl:16 * cl + 16, :].transpose(
                                             [1, 0, 2]))
                    # v tiles with ones column: (128, 34) per s_k chunk
                    vts = []
                    for kc in range(8):
                        vt = vtpool.tile([128, 34], F32R, tag="vones")
                        ci = 4 * b2 + kc // 2
                        half = kc % 2
                        nc.sync.dma_start(
                            out=vt[:].rearrange("p (m f) -> p m f",
                                                f=17)[:, :, 0:16],
                            in_=a2av_out[ci, 128 * half:128 * (half + 1),
                                         32 * g:32 * (g + 1)].rearrange(
                                             "p (m f) -> p m f", m=2))
                        nc.gpsimd.memset(vt[:, 16::17].bitcast(F32), 1.0)
                        vts.append(vt)

                    # stage: (17, [head m][q2][512]) unnormalized o^T + denoms
                    stg = dnpool.tile([17, 2 * S], F32R, tag="stg")
                    stg4 = stg[:].rearrange("p (m q f) -> p m q f", m=2, q=2)
                    for q2 in range(2):
                        av = avpsum.tile([17, 1024], F32, tag="av")
                        for kc in range(8):
                            sc = scpsum.tile([128, 1024], F32, tag="sc")
                            for m in range(2):
                                nc.tensor.matmul(
                                    sc[:, 512 * m:512 * (m + 1)],
                                    ks[32 * m:32 * m + 16,
                                       128 * kc:128 * (kc + 1)],
                                    qs[32 * m:32 * m + 16,
                                       512 * q2:512 * (q2 + 1)],
                                    start=True, stop=True,
                                    tile_position=(32 * m, 0))
                            ex = expool.tile([128, 1024], F32R, tag="ex")
                            nc.scalar.activation(ex[:], sc[:], AF.Exp, scale=0.25)
                            for m in range(2):
                                nc.tensor.matmul(
                                    av[:, 512 * m:512 * (m + 1)],
                                    vts[kc][:, 17 * m:17 * (m + 1)],
                                    ex[:, 512 * m:512 * (m + 1)],
                                    start=(kc == 0), stop=(kc == 7),
                                    skip_group_check=True)
                        nc.vector.tensor_copy(
                            stg4[:, :, q2, :],
                            av[:].rearrange("p (m f) -> p m f", m=2))
                    # export denominator rows; scatter o^T into packed tiles
                    for m in range(2):
                        cl = 2 * g + m
                        nc.sync.dma_start(out=rec_dram[b2, cl],
                                            in_=stg4[16:17, m].bitcast(F32))
                        on = on_tiles[(b2, cl // 4)]
                        nc.sync.dma_start(
                            out=on[32 * (cl % 4):32 * (cl % 4) + 16, :],
                            in_=stg4[0:16, m])

                # batched reciprocal of this batch-half's 8 denominator rows
                rt = dnpool.tile([8, S], F32, tag="rt")
                nc.sync.dma_start(out=rt[:], in_=rec_dram[b2])
                rt2 = dnpool.tile([8, S], F32, tag="rt2")
                nc.vector.reciprocal(rt2[:], rt[:])
                nc.sync.dma_start(out=rec2_dram[b2], in_=rt2[:])

        # ---------- phase 3: permutation + A2A + output projection ----------
        with tc.tile_pool(name="po", bufs=4) as popool, \
             tc.tile_pool(name="wo", bufs=3) as wopool, \
             tc.tile_pool(name="oo", bufs=1) as oopool:

            perm_sb = popool.tile([128, 32 * 128], F32R, tag="perm")
            nc.sync.dma_start(
                out=perm_sb[:].rearrange("p (n f) -> p n f", n=32),
                in_=perm.ap().transpose([1, 0, 2]))

            def psl(i):  # perm matrix slice index -> lhsT AP
                return perm_sb[:, 128 * i:128 * (i + 1)]

            for b2 in range(2):
                for v_ in range(4):
                    jp = ((v_ >> 1) & 1) * 4 + (v_ & 1) * 2 + b2
                    ps = ppsum.tile([128, 256], F32, tag="pp")
                    nmm = 0
                    for u in range(2):
                        src = on_tiles[(b2, u)]      # head cl=4u+v_ at rows 32v_
                        for r in range(4):
                            pi = (v_ * 4 + r) * 2 + u
                            nc.tensor.matmul(
                                ps[:],
                                psl(pi)[32 * v_:32 * v_ + 16, :],
                                src[32 * v_:32 * v_ + 16, r::4],
                                start=(nmm == 0), stop=(nmm == 7),
                                tile_position=(32 * v_, 0),
                                skip_group_check=True)
                            nmm += 1
                    # gathered reciprocal: M[64u+16r+h, a] = 1/denom[4u+v_, 4a+r]
                    mt = popool.tile([128, 256], F32, tag="mt")
                    for u in range(2):
                        for r in range(4):
                            src_ap = bass.AP(
                                tensor=rec2_dram,
                                offset=(8 * b2 + 4 * u + v_) * S + r,
                                ap=[[0, 16], [4, 256]])
                            nc.sync.dma_start(
                                out=mt[64 * u + 16 * r:64 * u + 16 * (r + 1), :],
                                in_=src_ap)
                    ot = popool.tile([128, 256], F32R, tag="pout")
                    nc.vector.tensor_mul(ot[:], ps[:], mt[:])
                    nc.sync.dma_start(out=a2ao_in[jp], in_=ot[:])
            a2a(a2ao_out, a2ao_in)

            rhs_o = []
            for k in range(KC):
                t = oopool.tile([128, ROWS], F32R, tag=f"ro{k}")
                nc.sync.dma_start(out=t[:], in_=a2ao_out[k])
                rhs_o.append(t)
            for n in range(KC):
                ps = ppsum.tile([128, ROWS], F32, tag="pp")
                wt = wopool.tile([128, KC * 128], F32R, tag="wo")
                nc.sync.dma_start(
                    out=wt[:].rearrange("p (k f) -> p k f", k=KC),
                    in_=W["wo_w"][:, n].transpose([1, 0, 2]))
                for k in range(KC):
                    nc.tensor.matmul(ps[:], wt[:, 128 * k:128 * (k + 1)],
                                     rhs_o[k][:],
                                     start=(k == 0), stop=(k == KC - 1))
                ot = oopool.tile([128, ROWS], F32, tag="fout")
                nc.vector.tensor_scalar_add(ot[:], ps[:], bcol["wo_b"][:, n:n + 1])
                nc.sync.dma_start(out=outT[128 * n:128 * (n + 1), :], in_=ot[:])

    nc.finalize()
    return nc


_NC_CACHE = None


def _get_nc():
    global _NC_CACHE
    if _NC_CACHE is None:
        _NC_CACHE = _build()
    return _NC_CACHE


def kernel(x, wq_w, wq_b, wk_w, wk_b, wv_w, wv_b,
           vq_w, vq_b, vk_w, vk_b, vv_w, vv_b, wo_w, wo_b,
           _trace=False):
    nc = _get_nc()

    ws = {"wq_w": wq_w, "vq_w": vq_w, "wk_w": wk_w, "vk_w": vk_w,
          "wv_w": wv_w, "vv_w": vv_w, "wo_w": wo_w}
    bs = {"wq_b": wq_b, "vq_b": vq_b, "wk_b": wk_b, "vk_b": vk_b,
          "wv_b": wv_b, "vv_b": vv_b, "wo_b": wo_b}

    wchunks = {n: np.ascontiguousarray(
        _round_tf32(np.asarray(w)).reshape(KC, 128, KC, 128).transpose(0, 2, 1, 3))
        for n, w in ws.items()}
    bmap = {n: np.ascontiguousarray(np.asarray(b, dtype=np.float32))
            for n, b in bs.items()}
    P = _round_tf32(_perm_mats())

    xf = np.asarray(x, dtype=np.float32).reshape(B * S, F)
    in_maps = []
    for j in range(NCORES):
        xTj = np.ascontiguousarray(
            _round_tf32(xf[ROWS * j:ROWS * (j + 1)]).T)
        m = {"xT": xTj, "perm": P}
        m.update(wchunks)
        m.update(bmap)
        in_maps.append(m)

    kw = {}
    if _trace:
        import sys
        import types
        if "antenv.axon_hooks" not in sys.modules:
            import antenv
            mod = types.ModuleType("antenv.axon_hooks")
            mod._hook = None
            def _set(h):
                mod._hook = h
            def _get():
                return mod._hook
            mod.set_axon_ntff_profile_hook = _set
            mod.get_axon_ntff_profile_hook = _get
            sys.modules["antenv.axon_hooks"] = mod
            antenv.axon_hooks = mod
            from trn_agent_boot.trn_boot import _ntff_profile_via_ctypes
            _set(_ntff_profile_via_ctypes("/opt/axon/libaxon_pjrt.so"))
        kw = dict(trace=True, trace_cores=list(range(NCORES)))
    res = run_bass_kernel_spmd(nc, in_maps, core_ids=list(range(NCORES)), **kw)

    out = np.empty((B * S, F), dtype=np.float32)
    for j in range(NCORES):
        out[ROWS * j:ROWS * (j + 1)] = res.results[j]["outT"].T
    if _trace:
        return out.reshape(B, S, F), res
    return out.reshape(B, S, F)


# revision 28
# speedup vs baseline: 1.7170x; 1.5166x over previous
"""Trainium2 Bass kernel for nn_MultiHeadAttention_90924457656943.

Strategy (8 NeuronCores, SPMD):
  - Row-shard the 2048 (b,s) token rows: core j owns rows [256j, 256j+256).
    Each core computes q/k/v for its rows with FULL weights (double
    projections done in transposed space: y^T = W^T @ x^T so weight chunks
    are the stationary matmul operand in natural layout).
  - AllToAll redistributes q/k/v so core j owns feature columns
    [128j, 128j+128) = effective heads [8j, 8j+8) for the full 2048 rows.
    q/k are shipped transposed (head-dim on partitions) which is exactly
    the layout attention needs; v ships natural.
  - Attention per (head, batch) with a flash-style loop: scores^T matmul
    (contraction over head_dim=16, packed 2 heads per pass into 32-row
    strips of the PE array), exp on ScalarE directly out of PSUM (the
    1/sqrt(16) scale folded into the activation's free affine), then
    attn@v with a ones-column appended to v so the softmax denominators
    fall out of the same matmul.
  - The module's quirky head-merge (torch .view after concat) is a fixed
    permutation; it maps each core's heads exactly onto o_perm columns
    [128j, 128j+128).  Constant 0/1 permutation matrices on the PE build
    the transposed o_perm^T slab directly.
  - AllToAll redistributes o_perm^T back to row shards; final projection
    runs transposed (out^T = wo^T @ o_perm^T) so the wo bias is a
    per-partition add.  Host transposes each core's (1024, 256) output
    shard back into rows.
  - All matmuls use float32r (TF32-like, full PE rate at N>=256); inputs
    are pre-rounded on the host.
"""

import os
import numpy as np

import concourse.bass as bass
import concourse.tile as tile
from concourse import bacc, mybir
from concourse.bass_utils import run_bass_kernel_spmd

F32 = mybir.dt.float32
F32R = mybir.dt.float32r
AF = mybir.ActivationFunctionType

B, S, F = 2, 1024, 1024
H = 16          # head dim
C = 64          # effective heads
NCORES = 8
ROWS = (B * S) // NCORES          # 256 token rows per core
KC = F // 128                     # 8 contraction chunks
HEADS_PER_CORE = C // NCORES      # 8


def _round_tf32(x: np.ndarray) -> np.ndarray:
    """Round fp32 to the PE's fp32r (tf32-like) format: RNE to 12 dropped bits."""
    u = np.ascontiguousarray(x, dtype=np.float32).view(np.uint32).copy()
    lsb = (u >> 12) & 1
    u += 0x7FF + lsb
    u &= np.uint32(0xFFFFF000)
    return u.view(np.float32)


def _perm_mats() -> np.ndarray:
    """32 constant matrices P[v,r,u]: rows 32v+h -> cols 64u+16r+h."""
    P = np.zeros((4, 4, 2, 128, 128), dtype=np.float32)
    for v in range(4):
        for r in range(4):
            for u in range(2):
                for h in range(H):
                    P[v, r, u, 32 * v + h, 64 * u + 16 * r + h] = 1.0
    return P.reshape(32, 128, 128)


WNAMES = ("wq_w", "vq_w", "wk_w", "vk_w", "wv_w", "vv_w", "wo_w")
BNAMES = ("wq_b", "vq_b", "wk_b", "vk_b", "wv_b", "vv_b", "wo_b")


def _build():
    nc = bacc.Bacc("TRN2", target_bir_lowering=False, debug=False,
                   num_devices=NCORES)

    xT = nc.dram_tensor("xT", [F, ROWS], F32R, kind="ExternalInput")
    # layout [m, p, k, j]: W[128k+p, 128m+j] -> contiguous (p, k*128+j) per m
    W = {n: nc.dram_tensor(n, [KC, 128, KC, 128], F32R, kind="ExternalInput")
         for n in WNAMES}
    vv_plain = nc.dram_tensor("vv_plain", [F, F], F32R, kind="ExternalInput")
    Bv = {n: nc.dram_tensor(n, [128, KC], F32, kind="ExternalInput")
          for n in BNAMES}
    b2vf = nc.dram_tensor("b2vf", [F], F32, kind="ExternalInput")
    perm = nc.dram_tensor("perm", [128, 32, 128], F32R, kind="ExternalInput")
    outT = nc.dram_tensor("outT", [F, ROWS], F32, kind="ExternalOutput")

    # internal DRAM: A2A bounce buffers + reciprocal broadcast bounce
    a2aq_in = nc.dram_tensor("a2aq_in", [NCORES, 128, ROWS], F32R)
    a2aq_out = nc.dram_tensor("a2aq_out", [NCORES, 128, ROWS], F32R)
    a2ak_in = nc.dram_tensor("a2ak_in", [NCORES, 128, ROWS], F32R)
    a2ak_out = nc.dram_tensor("a2ak_out", [NCORES, 128, ROWS], F32R)
    a2av_in = nc.dram_tensor("a2av_in", [NCORES, ROWS, 128], F32R)
    a2av_out = nc.dram_tensor("a2av_out", [NCORES, ROWS, 128], F32R)
    a2ao_in = nc.dram_tensor("a2ao_in", [NCORES, 128, ROWS], F32R)
    a2ao_out = nc.dram_tensor("a2ao_out", [NCORES, 128, ROWS], F32R)
    rec_dram = nc.dram_tensor("rec_dram", [2, 8, S], F32)   # (b2, 4u+v... flat row, s)
    rec2_dram = nc.dram_tensor("rec2_dram", [2, 8, 4, 256], F32)  # recip, (row, r, a)

    RG = [list(range(NCORES))]

    def a2a(dst, src):
        nc.gpsimd.collective_compute(
            "AllToAll", mybir.AluOpType.bypass,
            ins=[src[:]], outs=[dst[:]], replica_groups=RG)

    from contextlib import ExitStack
    with tile.TileContext(nc) as tc, ExitStack() as _stk:
        # ---------- persistent pools ----------
        const_pool = _stk.enter_context(tc.tile_pool(name="const", bufs=1))
        # biases as (128, 8) column tiles
        bcol = {}
        for n in BNAMES:
            t = const_pool.tile([128, KC], F32, tag=f"b_{n}")
            nc.sync.dma_start(out=t[:], in_=Bv[n].ap())
            bcol[n] = t
        # b2v broadcast across partitions (for the natural-layout v bias add)
        b2v_bc = const_pool.tile([128, F], F32, tag="b2v_bc")
        nc.sync.dma_start(out=b2v_bc[:], in_=b2vf.ap().partition_broadcast(128))

        # ---------- phase 1: projections ----------
        ppsum = _stk.enter_context(tc.tile_pool(name="ppsum", bufs=2, space="PSUM"))
        with tc.tile_pool(name="wpool", bufs=3) as wpool, \
             tc.tile_pool(name="wvpool", bufs=2) as wvpool, \
             tc.tile_pool(name="ypool", bufs=1) as ypool, \
             tc.tile_pool(name="stage", bufs=2) as stage:

            # x^T resident tiles (released with this pool after v's y1)
            xt = []
            for k in range(KC):
                t = ypool.tile([128, ROWS], F32R, tag=f"xt{k}")
                nc.sync.dma_start(out=t[:], in_=xT[128 * k:128 * (k + 1), :])
                xt.append(t)

            def projT(wname, bname, rhs_tiles, ytag):
                """y^T[mchunk] = sum_k W[k,m]^T-style matmul + bias; returns tiles."""
                out_tiles = []
                for m in range(KC):
                    ps = ppsum.tile([128, ROWS], F32, tag="pp")
                    wt = wpool.tile([128, KC * 128], F32R, tag="w")
                    nc.sync.dma_start(
                        out=wt[:].rearrange("p (k f) -> p k f", k=KC),
                        in_=W[wname][m])
                    for k in range(KC):
                        nc.tensor.matmul(ps[:], wt[:, 128 * k:128 * (k + 1)],
                                         rhs_tiles[k][:],
                                         start=(k == 0), stop=(k == KC - 1))
                    ot = ypool.tile([128, ROWS], F32R, tag=f"{ytag}{m}")
                    nc.vector.tensor_scalar_add(ot[:], ps[:], bcol[bname][:, m:m + 1])
                    out_tiles.append(ot)
                return out_tiles

            # q and k: two transposed projections, ship transposed
            for wn1, bn1, wn2, bn2, dst in (
                    ("wq_w", "wq_b", "vq_w", "vq_b", a2aq_in),
                    ("wk_w", "wk_b", "vk_w", "vk_b", a2ak_in)):
                y1 = projT(wn1, bn1, xt, "y1")
                y2 = projT(wn2, bn2, y1, "y2")
                for m in range(KC):
                    nc.sync.dma_start(out=dst[m], in_=y2[m][:])
                if dst is a2aq_in:
                    a2a(a2aq_out, a2aq_in)
                else:
                    a2a(a2ak_out, a2ak_in)

            # v: first projection transposed, second natural
            y1v = projT("wv_w", "wv_b", xt, "y1")
            for mb in range(ROWS // 128):          # bs chunk
                for n2 in range(F // 512):          # f_out 512-chunk
                    ps = ppsum.tile([128, 512], F32, tag="pp")
                    for k in range(KC):
                        wt = wvpool.tile([128, 512], F32R, tag="wv")
                        nc.sync.dma_start(
                            out=wt[:],
                            in_=vv_plain[128 * k:128 * (k + 1),
                                         512 * n2:512 * (n2 + 1)])
                        nc.tensor.matmul(
                            ps[:], y1v[k][:, 128 * mb:128 * (mb + 1)], wt[:],
                            start=(k == 0), stop=(k == KC - 1))
                    ot = stage.tile([128, 512], F32R, tag="vout")
                    nc.vector.tensor_add(ot[:], ps[:],
                                         b2v_bc[:, 512 * n2:512 * (n2 + 1)])
                    # scatter the 4 128-col chunks to their a2a slots
                    for mm in range(4):
                        nc.sync.dma_start(
                            out=a2av_in[4 * n2 + mm,
                                        128 * mb:128 * (mb + 1), :],
                            in_=ot[:, 128 * mm:128 * (mm + 1)])
            a2a(a2av_out, a2av_in)

        # ---------- phase 2: attention ----------
        # (b2, half) -> packed unnormalized o^T tile: head cl at rows 32*(cl%4)
        onpool = _stk.enter_context(tc.tile_pool(name="on", bufs=1))
        on_tiles = {}
        for _b2 in range(2):
            for _hf in range(2):
                on_t = onpool.tile([128, S], F32R, tag=f"on{2 * _b2 + _hf}")
                on_tiles[(_b2, _hf)] = on_t
        with tc.tile_pool(name="qk", bufs=2) as qkpool, \
             tc.tile_pool(name="vt", bufs=10) as vtpool, \
             tc.tile_pool(name="ex", bufs=2) as expool, \
             tc.tile_pool(name="dn", bufs=2) as dnpool, \
             tc.tile_pool(name="scp", bufs=2, space="PSUM") as scpsum, \
             tc.tile_pool(name="avp", bufs=1, space="PSUM") as avpsum:

            for b2 in range(2):
                for g in range(4):          # 2-head groups: heads 2g, 2g+1
                    qs = qkpool.tile([128, S], F32R, tag="qs")
                    ks = qkpool.tile([128, S], F32R, tag="ks")
                    for m in range(2):
                        cl = 2 * g + m
                        nc.sync.dma_start(
                            out=qs[32 * m:32 * m + 16, :].rearrange(
                                "p (i f) -> p i f", i=4),
                            in_=a2aq_out[4 * b2:4 * (b2 + 1),
                                         16 * cl:16 * cl + 16, :].transpose(
                                             [1, 0, 2]))
                        nc.sync.dma_start(
                            out=ks[32 * m:32 * m + 16, :].rearrange(
                                "p (i f) -> p i f", i=4),
                            in_=a2ak_out[4 * b2:4 * (b2 + 1),
                                         16 * cl:16 * cl + 16, :].transpose(
                                             [1, 0, 2]))
                    # v tiles with ones column: (128, 34) per s_k chunk
                    vts = []
                    for kc in range(8):
                        vt = vtpool.tile([128, 34], F32R, tag="vones")
                        ci = 4 * b2 + kc // 2
                        half = kc % 2
                        nc.sync.dma_start(
                            out=vt[:].rearrange("p (m f) -> p m f",
                                                f=17)[:, :, 0:16],
                            in_=a2av_out[ci, 128 * half:128 * (half + 1),
                                         32 * g:32 * (g + 1)].rearrange(
                                             "p (m f) -> p m f", m=2))
                        nc.gpsimd.memset(vt[:, 16::17].bitcast(F32), 1.0)
                        vts.append(vt)

                    # stage: (17, [head m][q2][512]) unnormalized o^T + denoms
                    stg = dnpool.tile([17, 2 * S], F32R, tag="stg")
                    stg4 = stg[:].rearrange("p (m q f) -> p m q f", m=2, q=2)
                    for q2 in range(2):
                        av = avpsum.tile([17, 1024], F32, tag="av")
                        for kc in range(8):
                            sc = scpsum.tile([128, 1024], F32, tag="sc")
                            for m in range(2):
                                nc.tensor.matmul(
                                    sc[:, 512 * m:512 * (m + 1)],
                                    ks[32 * m:32 * m + 16,
                                       128 * kc:128 * (kc + 1)],
                                    qs[32 * m:32 * m + 16,
                                       512 * q2:512 * (q2 + 1)],
                                    start=True, stop=True,
                                    tile_position=(32 * m, 0))
                            ex = expool.tile([128, 1024], F32R, tag="ex")
                            nc.scalar.activation(ex[:], sc[:], AF.Exp, scale=0.25)
                            for m in range(2):
                                nc.tensor.matmul(
                                    av[:, 512 * m:512 * (m + 1)],
                                    vts[kc][:, 17 * m:17 * (m + 1)],
                                    ex[:, 512 * m:512 * (m + 1)],
                                    start=(kc == 0), stop=(kc == 7),
                                    skip_group_check=True)
                        nc.vector.tensor_copy(
                            stg4[:, :, q2, :],
                            av[:].rearrange("p (m f) -> p m f", m=2))
                    # export denominator rows; scatter o^T into packed tiles
                    for m in range(2):
                        cl = 2 * g + m
                        nc.sync.dma_start(out=rec_dram[b2, cl],
                                            in_=stg4[16:17, m].bitcast(F32))
                        on = on_tiles[(b2, cl // 4)]
                        nc.sync.dma_start(
                            out=on[32 * (cl % 4):32 * (cl % 4) + 16, :],
                            in_=stg4[0:16, m])

                # batched reciprocal of this batch-half's 8 denominator rows
                rt = dnpool.tile([8, S], F32, tag="rt")
                nc.sync.dma_start(out=rt[:], in_=rec_dram[b2])
                rt2 = dnpool.tile([8, S], F32, tag="rt2")
                nc.vector.reciprocal(rt2[:], rt[:])
                # deinterleave (row, 4a+r) -> (row, r, a) so later reads are dense
                rt3 = dnpool.tile([8, S], F32, tag="rt3")
                nc.vector.tensor_copy(
                    rt3[:].rearrange("p (r a) -> p r a", r=4),
                    rt2[:].rearrange("p (a r) -> p r a", r=4))
                nc.sync.dma_start(out=rec2_dram[b2], in_=rt3[:].rearrange(
                    "p (r a) -> p r a", r=4))

        # ---------- phase 3: permutation + A2A + output projection ----------
        with tc.tile_pool(name="po", bufs=4) as popool, \
             tc.tile_pool(name="wo", bufs=3) as wopool, \
             tc.tile_pool(name="oo", bufs=1) as oopool:

            perm_sb = popool.tile([128, 32 * 128], F32R, tag="perm")
            nc.sync.dma_start(
                out=perm_sb[:].rearrange("p (n f) -> p n f", n=32),
                in_=perm.ap())

            def psl(i):  # perm matrix slice index -> lhsT AP
                return perm_sb[:, 128 * i:128 * (i + 1)]

            for b2 in range(2):
                for v_ in range(4):
                    jp = ((v_ >> 1) & 1) * 4 + (v_ & 1) * 2 + b2
                    ps = ppsum.tile([128, 256], F32, tag="pp")
                    nmm = 0
                    for u in range(2):
                        src = on_tiles[(b2, u)]      # head cl=4u+v_ at rows 32v_
                        for r in range(4):
                            pi = (v_ * 4 + r) * 2 + u
                            nc.tensor.matmul(
                                ps[:],
                                psl(pi)[32 * v_:32 * v_ + 16, :],
                                src[32 * v_:32 * v_ + 16, r::4],
                                start=(nmm == 0), stop=(nmm == 7),
                                tile_position=(32 * v_, 0),
                                skip_group_check=True)
                            nmm += 1
                    # gathered reciprocal: M[64u+16r+h, a] = 1/denom[4u+v_, 4a+r]
                    mt = popool.tile([128, 256], F32, tag="mt")
                    for u in range(2):
                        for r in range(4):
                            src_ap = bass.AP(
                                tensor=rec2_dram,
                                offset=(8 * b2 + 4 * u + v_) * S + r * 256,
                                ap=[[0, 16], [1, 256]])
                            nc.sync.dma_start(
                                out=mt[64 * u + 16 * r:
                                       64 * u + 16 * (r + 1), :],
                                in_=src_ap)
                    ot = popool.tile([128, 256], F32R, tag="pout")
                    nc.vector.tensor_mul(ot[:], ps[:], mt[:])
                    nc.sync.dma_start(out=a2ao_in[jp], in_=ot[:])
            a2a(a2ao_out, a2ao_in)

            rhs_o = []
            for k in range(KC):
                t = oopool.tile([128, ROWS], F32R, tag=f"ro{k}")
                nc.sync.dma_start(out=t[:], in_=a2ao_out[k])
                rhs_o.append(t)
            for n in range(KC):
                ps = ppsum.tile([128, ROWS], F32, tag="pp")
                wt = wopool.tile([128, KC * 128], F32R, tag="wo")
                nc.sync.dma_start(
                    out=wt[:].rearrange("p (k f) -> p k f", k=KC),
                    in_=W["wo_w"][n])
                for k in range(KC):
                    nc.tensor.matmul(ps[:], wt[:, 128 * k:128 * (k + 1)],
                                     rhs_o[k][:],
                                     start=(k == 0), stop=(k == KC - 1))
                ot = oopool.tile([128, ROWS], F32, tag="fout")
                nc.vector.tensor_scalar_add(ot[:], ps[:], bcol["wo_b"][:, n:n + 1])
                nc.sync.dma_start(out=outT[128 * n:128 * (n + 1), :], in_=ot[:])

    nc.finalize()
    return nc


_NC_CACHE = None


def _get_nc():
    global _NC_CACHE
    if _NC_CACHE is None:
        _NC_CACHE = _build()
    return _NC_CACHE


def kernel(x, wq_w, wq_b, wk_w, wk_b, wv_w, wv_b,
           vq_w, vq_b, vk_w, vk_b, vv_w, vv_b, wo_w, wo_b,
           _trace=False):
    nc = _get_nc()

    ws = {"wq_w": wq_w, "vq_w": vq_w, "wk_w": wk_w, "vk_w": vk_w,
          "wv_w": wv_w, "vv_w": vv_w, "wo_w": wo_w}
    bs = {"wq_b": wq_b, "vq_b": vq_b, "wk_b": wk_b, "vk_b": vk_b,
          "wv_b": wv_b, "vv_b": vv_b, "wo_b": wo_b}

    wchunks = {n: np.ascontiguousarray(
        _round_tf32(np.asarray(w)).reshape(KC, 128, KC, 128).transpose(2, 1, 0, 3))
        for n, w in ws.items()}
    bmap = {n: np.ascontiguousarray(
        np.asarray(b, dtype=np.float32).reshape(KC, 128).T)
            for n, b in bs.items()}
    b2vf_host = np.ascontiguousarray(np.asarray(vv_b, dtype=np.float32))
    vv_plain_host = _round_tf32(np.asarray(vv_w))
    P = np.ascontiguousarray(
        _round_tf32(_perm_mats()).transpose(1, 0, 2))

    xf = np.asarray(x, dtype=np.float32).reshape(B * S, F)
    in_maps = []
    for j in range(NCORES):
        xTj = np.ascontiguousarray(
            _round_tf32(xf[ROWS * j:ROWS * (j + 1)]).T)
        m = {"xT": xTj, "perm": P, "vv_plain": vv_plain_host,
             "b2vf": b2vf_host}
        m.update(wchunks)
        m.update(bmap)
        in_maps.append(m)

    kw = {}
    if _trace:
        import sys
        import types
        if "antenv.axon_hooks" not in sys.modules:
            import antenv
            mod = types.ModuleType("antenv.axon_hooks")
            mod._hook = None
            def _set(h):
                mod._hook = h
            def _get():
                return mod._hook
            mod.set_axon_ntff_profile_hook = _set
            mod.get_axon_ntff_profile_hook = _get
            sys.modules["antenv.axon_hooks"] = mod
            antenv.axon_hooks = mod
            from trn_agent_boot.trn_boot import _ntff_profile_via_ctypes
            _set(_ntff_profile_via_ctypes("/opt/axon/libaxon_pjrt.so"))
        kw = dict(trace=True, trace_cores=list(range(NCORES)))
    res = run_bass_kernel_spmd(nc, in_maps, core_ids=list(range(NCORES)), **kw)

    out = np.empty((B * S, F), dtype=np.float32)
    for j in range(NCORES):
        out[ROWS * j:ROWS * (j + 1)] = res.results[j]["outT"].T
    if _trace:
        return out.reshape(B, S, F), res
    return out.reshape(B, S, F)
